# revision 16
# baseline (speedup 1.0000x reference)
"""MoE top-1 router + expert MLPs on 8 Trainium2 NeuronCores.

Strategy (expert-parallel, per sharding hint):
  - Host computes the top-1 routing (argmax of softmax over a tiny [T,8]
    router matmul) and uses it to dispatch tokens: core e receives the
    tokens assigned to expert e (padded to a common capacity C), plus
    expert e's weights pre-tiled into the exact SBUF layouts the kernel
    consumes.
  - Each core runs fc1 -> exact gelu -> fc2 -> (+bias) * gate for its
    C tokens. Matmuls run as float32r (TF32-style fp32) at full PE rate.
    Both layers keep tokens as the moving operand (outputs stay
    feature-major), so PE work scales directly with C.
  - Host scatters each core's [H, C] output back to token order and
    computes the scalar load-balance aux loss.

All shapes are hardcoded for x: [4, 1024, 1024] f32, 8 experts,
hidden 1024, ffn 4096.
"""

import os
import sys

if "/opt/trn_rl_repo" not in sys.path:
    sys.path.insert(0, "/opt/trn_rl_repo")
# The axon PJRT backend must be discoverable; a pinned JAX_PLATFORMS=cpu
# would make run_bass_kernel_spmd run nowhere.
if os.environ.get("JAX_PLATFORMS") == "cpu" and "jax" not in sys.modules:
    os.environ["JAX_PLATFORMS"] = ""

import numpy as np

import concourse.bass as bass  # noqa: F401  (engine handles come via nc)
import concourse.tile as tile
from concourse import bacc, mybir
from concourse.bass_utils import run_bass_kernel_spmd
from concourse.tile_rust import add_dep_helper

P = 128
H = 1024
F = 4096
E = 8
LB_WEIGHT = 0.01

KO1 = H // P  # fc1 contraction K-tiles (8)
FC = F // P  # fc1 output F-tiles (32)
KO2 = F // P  # fc2 contraction K-tiles (32)
HO = H // P  # fc2 output h-tiles (8)

f32 = mybir.dt.float32
f32r = mybir.dt.float32r

# Stash of the most recent BassKernelResults (exec time / trace) for test.py.
LAST_RESULTS = None
_NC_CACHE: dict[int, object] = {}


def _c_chunks(C):
    """Split C into equal even chunks of width in [256, 512].

    Widths below 256 would drop float32r matmuls to 1/4 rate; odd widths
    violate the fp32r access-pattern rules.
    """
    n = -(-C // 512)
    while C % n or (C // n) % 2:
        n += 1
    w = C // n
    assert w >= 256, C
    return [(i * w, w) for i in range(n)]


def _build_nc(C):
    nc = bacc.Bacc("TRN2", target_bir_lowering=False, debug=False, num_devices=E)

    x_d = nc.dram_tensor("x_t", [P, KO1, C], f32, kind="ExternalInput").ap()
    w1_d = nc.dram_tensor("w1_t", [FC, P, KO1, P], f32, kind="ExternalInput").ap()
    b1_d = nc.dram_tensor("b1_t", [P, FC], f32, kind="ExternalInput").ap()
    w2_d = nc.dram_tensor("w2_t", [HO, P, KO2, P], f32, kind="ExternalInput").ap()
    b2_d = nc.dram_tensor("b2_t", [P, HO], f32, kind="ExternalInput").ap()
    g_d = nc.dram_tensor("g_t", [P, C], f32, kind="ExternalInput").ap()
    y_d = nc.dram_tensor("y", [H, C], f32, kind="ExternalOutput").ap()

    chunks = _c_chunks(C)

    with tile.TileContext(nc) as tc:
        with (
            tc.tile_pool(name="resident", bufs=1) as resident,
            tc.tile_pool(name="w1p", bufs=4) as w1p,
            tc.tile_pool(name="w2p", bufs=4 if C <= 576 else 3) as w2p,
            tc.tile_pool(name="outp", bufs=3) as outp,
            tc.tile_pool(name="ps1", bufs=3, space="PSUM") as ps1,
            tc.tile_pool(name="ps2", bufs=4, space="PSUM") as ps2,
        ):
            # The first fc1 matmuls are the critical path out of the
            # preamble: split w1[0] per-k and x[0] per-chunk so matmul (k=0,
            # chunk 0) waits on ~200KB instead of ~3MB.
            w1_first = [
                w1p.tile([P, P], f32r, tag=f"w1f{k}", name=f"w1f{k}")
                for k in range(KO1)
            ]
            x0_sb = []
            for c0, cw in _c_chunks(C):
                x0_sb.append(
                    resident.tile([P, cw], f32r, tag=f"x0_{c0}", name=f"x0_{c0}")
                )
            # issue order = time-to-first-matmul order: (k=0 weights, chunk-0
            # rhs) first, then the rest of iteration 0's operands
            nc.sync.dma_start(w1_first[0][:], w1_d[0, :, 0, :].bitcast(f32r))
            for ci, (c0, cw) in enumerate(_c_chunks(C)):
                nc.sync.dma_start(x0_sb[ci][:], x_d[:, 0, c0 : c0 + cw].bitcast(f32r))
            for k in range(1, KO1):
                nc.sync.dma_start(w1_first[k][:], w1_d[0, :, k, :].bitcast(f32r))
            # remaining x slices per-ko
            x_sb = [None]
            for k in range(1, KO1):
                xk = resident.tile([P, C], f32r, tag=f"x{k}")
                nc.sync.dma_start(xk[:], x_d[:, k, :].bitcast(f32r))
                x_sb.append(xk)
            b1_sb = resident.tile([P, FC], f32)
            nc.sync.dma_start(b1_sb[:], b1_d)
            b2_sb = resident.tile([P, HO], f32)
            nc.gpsimd.dma_start(b2_sb[:], b2_d)
            g_sb = resident.tile([P, C], f32)
            nc.gpsimd.dma_start(g_sb[:], g_d)
            h_sb = resident.tile([P, KO2, C], f32r)

            # fc2 weights: four 0.5MB DMAs per output h-tile. Each load is
            # anchored (real dependency edge) to a chosen fc1 gelu so the
            # scheduler cannot hoist the bulk w2 traffic into the early fc1
            # window where it would starve the w1 stream.
            w2_tiles: dict[int, object] = {}

            def load_w2(ho, anchor=None):
                if ho in w2_tiles or ho >= HO:
                    return
                t = w2p.tile([P, KO2, P], f32r, tag="w2")
                for g in range(4):
                    bi = nc.sync.dma_start(
                        t[:, g * 8 : (g + 1) * 8, :],
                        w2_d[ho][:, g * 8 : (g + 1) * 8, :].bitcast(f32r),
                    )
                    if anchor is not None:
                        add_dep_helper(
                            bi.ins, anchor.ins, reason="pace w2 prefetch vs fc1"
                        )
                w2_tiles[ho] = t

            # fc1: hT[f, c] = gelu(sum_k w1T[k, f] * xT[k, c] + b1[f])
            for fc in range(FC):
                if fc == 0:
                    w1_sb = None
                else:
                    w1_sb = w1p.tile([P, KO1, P], f32r)
                    nc.sync.dma_start(w1_sb[:], w1_d[fc].bitcast(f32r))
                for ci, (c0, cw) in enumerate(chunks):
                    ps = ps1.tile([P, 512], f32)
                    for k in range(KO1):
                        if fc == 0:
                            lhsT = w1_first[k][:]
                            rhs = (
                                x0_sb[ci][:, :]
                                if k == 0
                                else x_sb[k][:, c0 : c0 + cw]
                            )
                        else:
                            lhsT = w1_sb[:, k, :]
                            rhs = (
                                x0_sb[ci][:, :]
                                if k == 0
                                else x_sb[k][:, c0 : c0 + cw]
                            )
                        nc.tensor.matmul(
                            ps[:, :cw],
                            lhsT,
                            rhs,
                            start=(k == 0),
                            stop=(k == KO1 - 1),
                        )
                    act = nc.scalar.activation(
                        h_sb[:, fc, c0 : c0 + cw],
                        ps[:, :cw],
                        mybir.ActivationFunctionType.Gelu,
                        bias=b1_sb[:, fc : fc + 1],
                        scale=1.0,
                    )
                anchors = {10: 0, 20: 1, 27: 2, 31: 3}
                if fc in anchors:
                    load_w2(anchors[fc], anchor=act)

            # fc2: yT[h, c] = (sum_k w2T[k, h] * hT[k, c] + b2[h]) * g[c]
            for ho in range(HO):
                load_w2(ho)
                load_w2(ho + 4)  # slot-gated just-in-time prefetch
                w2_sb = w2_tiles[ho]
                for c0, cw in chunks:
                    ps = ps2.tile([P, 512], f32)
                    for k in range(KO2):
                        nc.tensor.matmul(
                            ps[:, :cw],
                            w2_sb[:, k, :],
                            h_sb[:, k, c0 : c0 + cw],
                            start=(k == 0),
                            stop=(k == KO2 - 1),
                        )
                    o1 = outp.tile([P, 512], f32, tag="o1")
                    nc.scalar.activation(
                        o1[:, :cw],
                        ps[:, :cw],
                        mybir.ActivationFunctionType.Identity,
                        bias=b2_sb[:, ho : ho + 1],
                        scale=1.0,
                    )
                    o2 = outp.tile([P, 512], f32, tag="o2")
                    nc.vector.tensor_mul(o2[:, :cw], o1[:, :cw], g_sb[:, c0 : c0 + cw])
                    nc.scalar.dma_start(
                        y_d[ho * P : (ho + 1) * P, c0 : c0 + cw], o2[:, :cw]
                    )

    nc.compile()
    return nc


def kernel(x, router_w, fc1_w, fc1_b, fc2_w, fc2_b):
    global LAST_RESULTS
    x = np.ascontiguousarray(np.asarray(x, dtype=np.float32))
    router_w = np.asarray(router_w, dtype=np.float32)
    B, S, _ = x.shape
    T = B * S
    xf = x.reshape(T, H)

    # --- host routing control plane (top-1 dispatch) ---
    logits = xf @ router_w.T  # [T, E] f32
    m = logits.max(-1, keepdims=True)
    ex = np.exp(logits - m, dtype=np.float32)
    denom = ex.sum(-1, dtype=np.float32)
    gates = np.float32(1.0) / denom  # max prob = exp(m - m) / denom
    idx = logits.argmax(-1)
    counts = np.bincount(idx, minlength=E)
    C = max(256, int(-(-counts.max() // 64) * 64))

    perm = [np.nonzero(idx == e)[0] for e in range(E)]

    if C not in _NC_CACHE:
        _NC_CACHE[C] = _build_nc(C)
    nc = _NC_CACHE[C]

    in_maps = []
    for e in range(E):
        pe = perm[e]
        ne = len(pe)
        xg = np.zeros((C, H), dtype=np.float32)
        xg[:ne] = xf[pe]
        x_t = np.ascontiguousarray(xg.T.reshape(KO1, P, C).transpose(1, 0, 2))
        w1 = np.asarray(fc1_w[e], dtype=np.float32)  # [F, H]
        w1_t = np.ascontiguousarray(
            w1.reshape(FC, P, KO1, P).transpose(0, 3, 2, 1)
        )  # [fc, kp, ko, fi]
        b1_t = np.ascontiguousarray(
            np.asarray(fc1_b[e], dtype=np.float32).reshape(FC, P).T
        )
        w2 = np.asarray(fc2_w[e], dtype=np.float32)  # [H, F]
        w2_t = np.ascontiguousarray(
            w2.reshape(HO, P, KO2, P).transpose(0, 3, 2, 1)
        )  # [ho, kp, ko, hi]
        b2_t = np.ascontiguousarray(
            np.asarray(fc2_b[e], dtype=np.float32).reshape(HO, P).T
        )
        ge = np.zeros(C, dtype=np.float32)
        ge[:ne] = gates[pe]
        g_t = np.ascontiguousarray(np.broadcast_to(ge, (P, C)))
        in_maps.append(
            {
                "x_t": x_t,
                "w1_t": w1_t,
                "b1_t": b1_t,
                "w2_t": w2_t,
                "b2_t": b2_t,
                "g_t": g_t,
            }
        )

    res = run_bass_kernel_spmd(nc, in_maps, core_ids=list(range(E)))
    LAST_RESULTS = res

    out_flat = np.empty((T, H), dtype=np.float32)
    for e in range(E):
        pe = perm[e]
        out_flat[pe] = res.results[e]["y"].T[: len(pe)]
    output = out_flat.reshape(B, S, H)

    # load-balance aux loss (scalar; part of the routing control plane)
    probs_sum = (ex / denom[:, None]).sum(axis=0, dtype=np.float64)
    mean_probs = probs_sum / T
    freq = counts.astype(np.float64) / T
    aux_loss = np.float32(LB_WEIGHT * E * np.sum(mean_probs * freq))
    return output, aux_loss


# revision 19
# speedup vs baseline: 1.0095x; 1.0095x over previous
"""MoE top-1 router + expert MLPs on 8 Trainium2 NeuronCores.

Strategy (expert-parallel, per sharding hint):
  - Host computes the top-1 routing (argmax of softmax over a tiny [T,8]
    router matmul) and uses it to dispatch tokens: core e receives the
    tokens assigned to expert e (padded to a common capacity C), plus
    expert e's weights pre-tiled into the exact SBUF layouts the kernel
    consumes.
  - Each core runs fc1 -> exact gelu -> fc2 -> (+bias) * gate for its
    C tokens. Matmuls run as float32r (TF32-style fp32) at full PE rate.
    Both layers keep tokens as the moving operand (outputs stay
    feature-major), so PE work scales directly with C.
  - Host scatters each core's [H, C] output back to token order and
    computes the scalar load-balance aux loss.

All shapes are hardcoded for x: [4, 1024, 1024] f32, 8 experts,
hidden 1024, ffn 4096.
"""

import os
import sys

if "/opt/trn_rl_repo" not in sys.path:
    sys.path.insert(0, "/opt/trn_rl_repo")
# The axon PJRT backend must be discoverable; a pinned JAX_PLATFORMS=cpu
# would make run_bass_kernel_spmd run nowhere.
if os.environ.get("JAX_PLATFORMS") == "cpu" and "jax" not in sys.modules:
    os.environ["JAX_PLATFORMS"] = ""

import numpy as np

import concourse.bass as bass  # noqa: F401  (engine handles come via nc)
import concourse.tile as tile
from concourse import bacc, mybir
from concourse.bass_utils import run_bass_kernel_spmd
from concourse.tile_rust import add_dep_helper

P = 128
H = 1024
F = 4096
E = 8
LB_WEIGHT = 0.01

KO1 = H // P  # fc1 contraction K-tiles (8)
FC = F // P  # fc1 output F-tiles (32)
KO2 = F // P  # fc2 contraction K-tiles (32)
HO = H // P  # fc2 output h-tiles (8)

f32 = mybir.dt.float32
f32r = mybir.dt.float32r

# Stash of the most recent BassKernelResults (exec time / trace) for test.py.
LAST_RESULTS = None
_NC_CACHE: dict[int, object] = {}


def _c_chunks(C):
    """Split C into equal even chunks of width in [256, 512].

    Widths below 256 would drop float32r matmuls to 1/4 rate; odd widths
    violate the fp32r access-pattern rules.
    """
    n = -(-C // 512)
    while C % n or (C // n) % 2:
        n += 1
    w = C // n
    assert w >= 256, C
    return [(i * w, w) for i in range(n)]


def _build_nc(C):
    nc = bacc.Bacc("TRN2", target_bir_lowering=False, debug=False, num_devices=E)

    x_d = nc.dram_tensor("x_t", [P, KO1, C], f32, kind="ExternalInput").ap()
    w1_d = nc.dram_tensor("w1_t", [FC, P, KO1, P], f32, kind="ExternalInput").ap()
    b1_d = nc.dram_tensor("b1_t", [P, FC], f32, kind="ExternalInput").ap()
    w2_d = nc.dram_tensor("w2_t", [HO, P, KO2, P], f32, kind="ExternalInput").ap()
    b2_d = nc.dram_tensor("b2_t", [P, HO], f32, kind="ExternalInput").ap()
    g_d = nc.dram_tensor("g_t", [P, C], f32, kind="ExternalInput").ap()
    y_d = nc.dram_tensor("y", [H, C], f32, kind="ExternalOutput").ap()

    chunks = _c_chunks(C)

    with tile.TileContext(nc) as tc:
        with (
            tc.tile_pool(name="resident", bufs=1) as resident,
            tc.tile_pool(name="w1p", bufs=4) as w1p,
            tc.tile_pool(name="w2p", bufs=4 if C <= 576 else 3) as w2p,
            tc.tile_pool(name="outp", bufs=3) as outp,
            tc.tile_pool(name="ps1", bufs=3, space="PSUM") as ps1,
            tc.tile_pool(name="ps2", bufs=4, space="PSUM") as ps2,
        ):
            # Warm the PE HAM clock-gate during the DMA fill window:
            # ~7us of matmuls on zeroed tiles unthrottles the clock
            # (1.2 -> 2.4 GHz needs ~3.4us of sustained PE activity), so
            # the real fc1 stream starts warm. Results are discarded.
            bf16 = mybir.dt.bfloat16
            wz_s = resident.tile([P, P], bf16, tag="wz_s")
            wz_m = resident.tile([P, 512], bf16, tag="wz_m")
            nc.vector.memset(wz_s[:], 0)
            nc.vector.memset(wz_m[:], 0)
            ps_w = ps1.tile([P, 512], f32, tag="ps", name="ps_warm")
            for i in range(14):
                nc.tensor.matmul(
                    ps_w[:], wz_s[:], wz_m[:], start=(i == 0), stop=(i == 13)
                )

            # The first fc1 matmuls are the critical path out of the
            # preamble: split w1[0] per-k and x[0] per-chunk so matmul (k=0,
            # chunk 0) waits on ~200KB instead of ~3MB.
            w1_first = [
                w1p.tile([P, P], f32r, tag=f"w1f{k}", name=f"w1f{k}")
                for k in range(KO1)
            ]
            x0_sb = []
            for c0, cw in _c_chunks(C):
                x0_sb.append(
                    resident.tile([P, cw], f32r, tag=f"x0_{c0}", name=f"x0_{c0}")
                )
            # issue order = time-to-first-matmul order: (k=0 weights, chunk-0
            # rhs) first, then the rest of iteration 0's operands
            nc.sync.dma_start(w1_first[0][:], w1_d[0, :, 0, :].bitcast(f32r))
            for ci, (c0, cw) in enumerate(_c_chunks(C)):
                nc.sync.dma_start(x0_sb[ci][:], x_d[:, 0, c0 : c0 + cw].bitcast(f32r))
            for k in range(1, KO1):
                nc.sync.dma_start(w1_first[k][:], w1_d[0, :, k, :].bitcast(f32r))
            # remaining x slices per-ko
            x_sb = [None]
            for k in range(1, KO1):
                xk = resident.tile([P, C], f32r, tag=f"x{k}")
                nc.sync.dma_start(xk[:], x_d[:, k, :].bitcast(f32r))
                x_sb.append(xk)
            b1_sb = resident.tile([P, FC], f32)
            nc.sync.dma_start(b1_sb[:], b1_d)
            b2_sb = resident.tile([P, HO], f32)
            nc.gpsimd.dma_start(b2_sb[:], b2_d)
            g_sb = resident.tile([P, C], f32)
            nc.gpsimd.dma_start(g_sb[:], g_d)
            h_sb = resident.tile([P, KO2, C], f32r)

            # fc2 weights: four 0.5MB DMAs per output h-tile. Each load is
            # anchored (real dependency edge) to a chosen fc1 gelu so the
            # scheduler cannot hoist the bulk w2 traffic into the early fc1
            # window where it would starve the w1 stream.
            w2_tiles: dict[int, object] = {}

            def load_w2(ho, anchor=None):
                if ho in w2_tiles or ho >= HO:
                    return
                t = w2p.tile([P, KO2, P], f32r, tag="w2")
                for g in range(4):
                    bi = nc.sync.dma_start(
                        t[:, g * 8 : (g + 1) * 8, :],
                        w2_d[ho][:, g * 8 : (g + 1) * 8, :].bitcast(f32r),
                    )
                    if anchor is not None:
                        add_dep_helper(
                            bi.ins, anchor.ins, reason="pace w2 prefetch vs fc1"
                        )
                w2_tiles[ho] = t

            # fc1: hT[f, c] = gelu(sum_k w1T[k, f] * xT[k, c] + b1[f])
            for fc in range(FC):
                if fc == 0:
                    w1_sb = None
                else:
                    w1_sb = w1p.tile([P, KO1, P], f32r)
                    nc.sync.dma_start(w1_sb[:], w1_d[fc].bitcast(f32r))
                for ci, (c0, cw) in enumerate(chunks):
                    ps = ps1.tile([P, 512], f32)
                    for k in range(KO1):
                        if fc == 0:
                            lhsT = w1_first[k][:]
                            rhs = (
                                x0_sb[ci][:, :]
                                if k == 0
                                else x_sb[k][:, c0 : c0 + cw]
                            )
                        else:
                            lhsT = w1_sb[:, k, :]
                            rhs = (
                                x0_sb[ci][:, :]
                                if k == 0
                                else x_sb[k][:, c0 : c0 + cw]
                            )
                        nc.tensor.matmul(
                            ps[:, :cw],
                            lhsT,
                            rhs,
                            start=(k == 0),
                            stop=(k == KO1 - 1),
                        )
                    act = nc.scalar.activation(
                        h_sb[:, fc, c0 : c0 + cw],
                        ps[:, :cw],
                        mybir.ActivationFunctionType.Gelu,
                        bias=b1_sb[:, fc : fc + 1],
                        scale=1.0,
                    )
                anchors = {10: 0, 20: 1, 27: 2, 31: 3}
                if fc in anchors:
                    load_w2(anchors[fc], anchor=act)

            # fc2: yT[h, c] = (sum_k w2T[k, h] * hT[k, c] + b2[h]) * g[c]
            for ho in range(HO):
                load_w2(ho)
                load_w2(ho + 4)  # slot-gated just-in-time prefetch
                w2_sb = w2_tiles[ho]
                for c0, cw in chunks:
                    ps = ps2.tile([P, 512], f32)
                    for k in range(KO2):
                        nc.tensor.matmul(
                            ps[:, :cw],
                            w2_sb[:, k, :],
                            h_sb[:, k, c0 : c0 + cw],
                            start=(k == 0),
                            stop=(k == KO2 - 1),
                        )
                    o1 = outp.tile([P, 512], f32, tag="o1")
                    nc.scalar.activation(
                        o1[:, :cw],
                        ps[:, :cw],
                        mybir.ActivationFunctionType.Identity,
                        bias=b2_sb[:, ho : ho + 1],
                        scale=1.0,
                    )
                    o2 = outp.tile([P, 512], f32, tag="o2")
                    nc.vector.tensor_mul(o2[:, :cw], o1[:, :cw], g_sb[:, c0 : c0 + cw])
                    nc.scalar.dma_start(
                        y_d[ho * P : (ho + 1) * P, c0 : c0 + cw], o2[:, :cw]
                    )

    nc.compile()
    return nc


def kernel(x, router_w, fc1_w, fc1_b, fc2_w, fc2_b):
    global LAST_RESULTS
    x = np.ascontiguousarray(np.asarray(x, dtype=np.float32))
    router_w = np.asarray(router_w, dtype=np.float32)
    B, S, _ = x.shape
    T = B * S
    xf = x.reshape(T, H)

    # --- host routing control plane (top-1 dispatch) ---
    logits = xf @ router_w.T  # [T, E] f32
    m = logits.max(-1, keepdims=True)
    ex = np.exp(logits - m, dtype=np.float32)
    denom = ex.sum(-1, dtype=np.float32)
    gates = np.float32(1.0) / denom  # max prob = exp(m - m) / denom
    idx = logits.argmax(-1)
    counts = np.bincount(idx, minlength=E)
    C = max(256, int(-(-counts.max() // 64) * 64))

    perm = [np.nonzero(idx == e)[0] for e in range(E)]

    if C not in _NC_CACHE:
        _NC_CACHE[C] = _build_nc(C)
    nc = _NC_CACHE[C]

    in_maps = []
    for e in range(E):
        pe = perm[e]
        ne = len(pe)
        xg = np.zeros((C, H), dtype=np.float32)
        xg[:ne] = xf[pe]
        x_t = np.ascontiguousarray(xg.T.reshape(KO1, P, C).transpose(1, 0, 2))
        w1 = np.asarray(fc1_w[e], dtype=np.float32)  # [F, H]
        w1_t = np.ascontiguousarray(
            w1.reshape(FC, P, KO1, P).transpose(0, 3, 2, 1)
        )  # [fc, kp, ko, fi]
        b1_t = np.ascontiguousarray(
            np.asarray(fc1_b[e], dtype=np.float32).reshape(FC, P).T
        )
        w2 = np.asarray(fc2_w[e], dtype=np.float32)  # [H, F]
        w2_t = np.ascontiguousarray(
            w2.reshape(HO, P, KO2, P).transpose(0, 3, 2, 1)
        )  # [ho, kp, ko, hi]
        b2_t = np.ascontiguousarray(
            np.asarray(fc2_b[e], dtype=np.float32).reshape(HO, P).T
        )
        ge = np.zeros(C, dtype=np.float32)
        ge[:ne] = gates[pe]
        g_t = np.ascontiguousarray(np.broadcast_to(ge, (P, C)))
        in_maps.append(
            {
                "x_t": x_t,
                "w1_t": w1_t,
                "b1_t": b1_t,
                "w2_t": w2_t,
                "b2_t": b2_t,
                "g_t": g_t,
            }
        )

    res = run_bass_kernel_spmd(nc, in_maps, core_ids=list(range(E)))
    LAST_RESULTS = res

    out_flat = np.empty((T, H), dtype=np.float32)
    for e in range(E):
        pe = perm[e]
        out_flat[pe] = res.results[e]["y"].T[: len(pe)]
    output = out_flat.reshape(B, S, H)

    # load-balance aux loss (scalar; part of the routing control plane)
    probs_sum = (ex / denom[:, None]).sum(axis=0, dtype=np.float64)
    mean_probs = probs_sum / T
    freq = counts.astype(np.float64) / T
    aux_loss = np.float32(LB_WEIGHT * E * np.sum(mean_probs * freq))
    return output, aux_loss


# revision 20
# speedup vs baseline: 1.0189x; 1.0093x over previous
"""MoE top-1 router + expert MLPs on 8 Trainium2 NeuronCores.

Strategy (expert-parallel, per sharding hint):
  - Host computes the top-1 routing (argmax of softmax over a tiny [T,8]
    router matmul) and uses it to dispatch tokens: core e receives the
    tokens assigned to expert e (padded to a common capacity C), plus
    expert e's weights pre-tiled into the exact SBUF layouts the kernel
    consumes.
  - Each core runs fc1 -> exact gelu -> fc2 -> (+bias) * gate for its
    C tokens. Matmuls run as float32r (TF32-style fp32) at full PE rate.
    Both layers keep tokens as the moving operand (outputs stay
    feature-major), so PE work scales directly with C.
  - Host scatters each core's [H, C] output back to token order and
    computes the scalar load-balance aux loss.

All shapes are hardcoded for x: [4, 1024, 1024] f32, 8 experts,
hidden 1024, ffn 4096.
"""

import os
import sys

if "/opt/trn_rl_repo" not in sys.path:
    sys.path.insert(0, "/opt/trn_rl_repo")
# The axon PJRT backend must be discoverable; a pinned JAX_PLATFORMS=cpu
# would make run_bass_kernel_spmd run nowhere.
if os.environ.get("JAX_PLATFORMS") == "cpu" and "jax" not in sys.modules:
    os.environ["JAX_PLATFORMS"] = ""

import numpy as np

import concourse.bass as bass  # noqa: F401  (engine handles come via nc)
import concourse.tile as tile
from concourse import bacc, mybir
from concourse.bass_utils import run_bass_kernel_spmd
from concourse.tile_rust import add_dep_helper

P = 128
H = 1024
F = 4096
E = 8
LB_WEIGHT = 0.01

KO1 = H // P  # fc1 contraction K-tiles (8)
FC = F // P  # fc1 output F-tiles (32)
KO2 = F // P  # fc2 contraction K-tiles (32)
HO = H // P  # fc2 output h-tiles (8)

f32 = mybir.dt.float32
f32r = mybir.dt.float32r

# Stash of the most recent BassKernelResults (exec time / trace) for test.py.
LAST_RESULTS = None
_NC_CACHE: dict[int, object] = {}


def _c_chunks(C):
    """Split C into equal even chunks of width in [256, 512].

    Widths below 256 would drop float32r matmuls to 1/4 rate; odd widths
    violate the fp32r access-pattern rules.
    """
    n = -(-C // 512)
    while C % n or (C // n) % 2:
        n += 1
    w = C // n
    assert w >= 256, C
    return [(i * w, w) for i in range(n)]


def _build_nc(C):
    nc = bacc.Bacc("TRN2", target_bir_lowering=False, debug=False, num_devices=E)

    x_d = nc.dram_tensor("x_t", [P, KO1, C], f32, kind="ExternalInput").ap()
    w1_d = nc.dram_tensor("w1_t", [FC, P, KO1, P], f32, kind="ExternalInput").ap()
    b1_d = nc.dram_tensor("b1_t", [P, FC], f32, kind="ExternalInput").ap()
    w2_d = nc.dram_tensor("w2_t", [HO, P, KO2, P], f32, kind="ExternalInput").ap()
    b2_d = nc.dram_tensor("b2_t", [P, HO], f32, kind="ExternalInput").ap()
    g_d = nc.dram_tensor("g_t", [P, C], f32, kind="ExternalInput").ap()
    y_d = nc.dram_tensor("y", [H, C], f32, kind="ExternalOutput").ap()

    chunks = _c_chunks(C)

    with tile.TileContext(nc) as tc:
        with (
            tc.tile_pool(name="resident", bufs=1) as resident,
            tc.tile_pool(name="w1p", bufs=6) as w1p,
            tc.tile_pool(name="w2p", bufs=3) as w2p,
            tc.tile_pool(name="outp", bufs=3) as outp,
            tc.tile_pool(name="ps1", bufs=3, space="PSUM") as ps1,
            tc.tile_pool(name="ps2", bufs=4, space="PSUM") as ps2,
        ):
            # Warm the PE HAM clock-gate during the DMA fill window:
            # ~7us of matmuls on zeroed tiles unthrottles the clock
            # (1.2 -> 2.4 GHz needs ~3.4us of sustained PE activity), so
            # the real fc1 stream starts warm. Results are discarded.
            bf16 = mybir.dt.bfloat16
            wz_s = resident.tile([P, P], bf16, tag="wz_s")
            wz_m = resident.tile([P, 512], bf16, tag="wz_m")
            nc.vector.memset(wz_s[:], 0)
            nc.vector.memset(wz_m[:], 0)
            ps_w = ps1.tile([P, 512], f32, tag="ps", name="ps_warm")
            for i in range(14):
                nc.tensor.matmul(
                    ps_w[:], wz_s[:], wz_m[:], start=(i == 0), stop=(i == 13)
                )

            # The first fc1 matmuls are the critical path out of the
            # preamble: split w1[0] per-k and x[0] per-chunk so matmul (k=0,
            # chunk 0) waits on ~200KB instead of ~3MB.
            w1_first = [
                w1p.tile([P, P], f32r, tag=f"w1f{k}", name=f"w1f{k}")
                for k in range(KO1)
            ]
            x0_sb = []
            for c0, cw in _c_chunks(C):
                x0_sb.append(
                    resident.tile([P, cw], f32r, tag=f"x0_{c0}", name=f"x0_{c0}")
                )
            # issue order = time-to-first-matmul order: (k=0 weights, chunk-0
            # rhs) first, then the rest of iteration 0's operands
            nc.sync.dma_start(w1_first[0][:], w1_d[0, :, 0, :].bitcast(f32r))
            for ci, (c0, cw) in enumerate(_c_chunks(C)):
                nc.sync.dma_start(x0_sb[ci][:], x_d[:, 0, c0 : c0 + cw].bitcast(f32r))
            for k in range(1, KO1):
                nc.sync.dma_start(w1_first[k][:], w1_d[0, :, k, :].bitcast(f32r))
            # remaining x slices per-ko
            x_sb = [None]
            for k in range(1, KO1):
                xk = resident.tile([P, C], f32r, tag=f"x{k}")
                nc.sync.dma_start(xk[:], x_d[:, k, :].bitcast(f32r))
                x_sb.append(xk)
            b1_sb = resident.tile([P, FC], f32)
            nc.sync.dma_start(b1_sb[:], b1_d)
            b2_sb = resident.tile([P, HO], f32)
            nc.gpsimd.dma_start(b2_sb[:], b2_d)
            g_sb = resident.tile([P, C], f32)
            nc.gpsimd.dma_start(g_sb[:], g_d)
            h_sb = resident.tile([P, KO2, C], f32r)

            # fc2 weights: four 0.5MB DMAs per output h-tile. Each load is
            # anchored (real dependency edge) to a chosen fc1 gelu so the
            # scheduler cannot hoist the bulk w2 traffic into the early fc1
            # window where it would starve the w1 stream.
            w2_tiles: dict[int, object] = {}

            def load_w2(ho, anchor=None):
                if ho in w2_tiles or ho >= HO:
                    return
                t = w2p.tile([P, KO2, P], f32r, tag="w2")
                for g in range(4):
                    bi = nc.sync.dma_start(
                        t[:, g * 8 : (g + 1) * 8, :],
                        w2_d[ho][:, g * 8 : (g + 1) * 8, :].bitcast(f32r),
                    )
                    if anchor is not None:
                        add_dep_helper(
                            bi.ins, anchor.ins, reason="pace w2 prefetch vs fc1"
                        )
                w2_tiles[ho] = t

            # fc1: hT[f, c] = gelu(sum_k w1T[k, f] * xT[k, c] + b1[f])
            for fc in range(FC):
                if fc == 0:
                    w1_sb = None
                else:
                    w1_sb = w1p.tile([P, KO1, P], f32r)
                    nc.sync.dma_start(w1_sb[:], w1_d[fc].bitcast(f32r))
                for ci, (c0, cw) in enumerate(chunks):
                    ps = ps1.tile([P, 512], f32)
                    for k in range(KO1):
                        if fc == 0:
                            lhsT = w1_first[k][:]
                            rhs = (
                                x0_sb[ci][:, :]
                                if k == 0
                                else x_sb[k][:, c0 : c0 + cw]
                            )
                        else:
                            lhsT = w1_sb[:, k, :]
                            rhs = (
                                x0_sb[ci][:, :]
                                if k == 0
                                else x_sb[k][:, c0 : c0 + cw]
                            )
                        nc.tensor.matmul(
                            ps[:, :cw],
                            lhsT,
                            rhs,
                            start=(k == 0),
                            stop=(k == KO1 - 1),
                        )
                    act = nc.scalar.activation(
                        h_sb[:, fc, c0 : c0 + cw],
                        ps[:, :cw],
                        mybir.ActivationFunctionType.Gelu,
                        bias=b1_sb[:, fc : fc + 1],
                        scale=1.0,
                    )
                anchors = {10: 0, 20: 1, 27: 2, 31: 3}
                if fc in anchors:
                    load_w2(anchors[fc], anchor=act)

            # fc2: yT[h, c] = (sum_k w2T[k, h] * hT[k, c] + b2[h]) * g[c]
            for ho in range(HO):
                load_w2(ho)
                load_w2(ho + 4)  # slot-gated just-in-time prefetch
                w2_sb = w2_tiles[ho]
                for c0, cw in chunks:
                    ps = ps2.tile([P, 512], f32)
                    for k in range(KO2):
                        nc.tensor.matmul(
                            ps[:, :cw],
                            w2_sb[:, k, :],
                            h_sb[:, k, c0 : c0 + cw],
                            start=(k == 0),
                            stop=(k == KO2 - 1),
                        )
                    o1 = outp.tile([P, 512], f32, tag="o1")
                    nc.scalar.activation(
                        o1[:, :cw],
                        ps[:, :cw],
                        mybir.ActivationFunctionType.Identity,
                        bias=b2_sb[:, ho : ho + 1],
                        scale=1.0,
                    )
                    o2 = outp.tile([P, 512], f32, tag="o2")
                    nc.vector.tensor_mul(o2[:, :cw], o1[:, :cw], g_sb[:, c0 : c0 + cw])
                    nc.scalar.dma_start(
                        y_d[ho * P : (ho + 1) * P, c0 : c0 + cw], o2[:, :cw]
                    )

    nc.compile()
    return nc


def kernel(x, router_w, fc1_w, fc1_b, fc2_w, fc2_b):
    global LAST_RESULTS
    x = np.ascontiguousarray(np.asarray(x, dtype=np.float32))
    router_w = np.asarray(router_w, dtype=np.float32)
    B, S, _ = x.shape
    T = B * S
    xf = x.reshape(T, H)

    # --- host routing control plane (top-1 dispatch) ---
    logits = xf @ router_w.T  # [T, E] f32
    m = logits.max(-1, keepdims=True)
    ex = np.exp(logits - m, dtype=np.float32)
    denom = ex.sum(-1, dtype=np.float32)
    gates = np.float32(1.0) / denom  # max prob = exp(m - m) / denom
    idx = logits.argmax(-1)
    counts = np.bincount(idx, minlength=E)
    C = max(256, int(-(-counts.max() // 64) * 64))

    perm = [np.nonzero(idx == e)[0] for e in range(E)]

    if C not in _NC_CACHE:
        _NC_CACHE[C] = _build_nc(C)
    nc = _NC_CACHE[C]

    in_maps = []
    for e in range(E):
        pe = perm[e]
        ne = len(pe)
        xg = np.zeros((C, H), dtype=np.float32)
        xg[:ne] = xf[pe]
        x_t = np.ascontiguousarray(xg.T.reshape(KO1, P, C).transpose(1, 0, 2))
        w1 = np.asarray(fc1_w[e], dtype=np.float32)  # [F, H]
        w1_t = np.ascontiguousarray(
            w1.reshape(FC, P, KO1, P).transpose(0, 3, 2, 1)
        )  # [fc, kp, ko, fi]
        b1_t = np.ascontiguousarray(
            np.asarray(fc1_b[e], dtype=np.float32).reshape(FC, P).T
        )
        w2 = np.asarray(fc2_w[e], dtype=np.float32)  # [H, F]
        w2_t = np.ascontiguousarray(
            w2.reshape(HO, P, KO2, P).transpose(0, 3, 2, 1)
        )  # [ho, kp, ko, hi]
        b2_t = np.ascontiguousarray(
            np.asarray(fc2_b[e], dtype=np.float32).reshape(HO, P).T
        )
        ge = np.zeros(C, dtype=np.float32)
        ge[:ne] = gates[pe]
        g_t = np.ascontiguousarray(np.broadcast_to(ge, (P, C)))
        in_maps.append(
            {
                "x_t": x_t,
                "w1_t": w1_t,
                "b1_t": b1_t,
                "w2_t": w2_t,
                "b2_t": b2_t,
                "g_t": g_t,
            }
        )

    res = run_bass_kernel_spmd(nc, in_maps, core_ids=list(range(E)))
    LAST_RESULTS = res

    out_flat = np.empty((T, H), dtype=np.float32)
    for e in range(E):
        pe = perm[e]
        out_flat[pe] = res.results[e]["y"].T[: len(pe)]
    output = out_flat.reshape(B, S, H)

    # load-balance aux loss (scalar; part of the routing control plane)
    probs_sum = (ex / denom[:, None]).sum(axis=0, dtype=np.float64)
    mean_probs = probs_sum / T
    freq = counts.astype(np.float64) / T
    aux_loss = np.float32(LB_WEIGHT * E * np.sum(mean_probs * freq))
    return output, aux_loss


# revision 21
# speedup vs baseline: 1.0394x; 1.0201x over previous
"""MoE top-1 router + expert MLPs on 8 Trainium2 NeuronCores.

Strategy (expert-parallel, per sharding hint):
  - Host computes the top-1 routing (argmax of softmax over a tiny [T,8]
    router matmul) and uses it to dispatch tokens: core e receives the
    tokens assigned to expert e (padded to a common capacity C), plus
    expert e's weights pre-tiled into the exact SBUF layouts the kernel
    consumes.
  - Each core runs fc1 -> exact gelu -> fc2 -> (+bias) * gate for its
    C tokens. Matmuls run as float32r (TF32-style fp32) at full PE rate.
    Both layers keep tokens as the moving operand (outputs stay
    feature-major), so PE work scales directly with C.
  - Host scatters each core's [H, C] output back to token order and
    computes the scalar load-balance aux loss.

All shapes are hardcoded for x: [4, 1024, 1024] f32, 8 experts,
hidden 1024, ffn 4096.
"""

import os
import sys

if "/opt/trn_rl_repo" not in sys.path:
    sys.path.insert(0, "/opt/trn_rl_repo")
# The axon PJRT backend must be discoverable; a pinned JAX_PLATFORMS=cpu
# would make run_bass_kernel_spmd run nowhere.
if os.environ.get("JAX_PLATFORMS") == "cpu" and "jax" not in sys.modules:
    os.environ["JAX_PLATFORMS"] = ""

import numpy as np

import concourse.bass as bass  # noqa: F401  (engine handles come via nc)
import concourse.tile as tile
from concourse import bacc, mybir
from concourse.bass_utils import run_bass_kernel_spmd
from concourse.tile_rust import add_dep_helper

P = 128
H = 1024
F = 4096
E = 8
LB_WEIGHT = 0.01

KO1 = H // P  # fc1 contraction K-tiles (8)
FC = F // P  # fc1 output F-tiles (32)
KO2 = F // P  # fc2 contraction K-tiles (32)
HO = H // P  # fc2 output h-tiles (8)

f32 = mybir.dt.float32
f32r = mybir.dt.float32r

# Stash of the most recent BassKernelResults (exec time / trace) for test.py.
LAST_RESULTS = None
_NC_CACHE: dict[int, object] = {}


def _c_chunks(C):
    """Split C into equal even chunks of width in [256, 512].

    Widths below 256 would drop float32r matmuls to 1/4 rate; odd widths
    violate the fp32r access-pattern rules.
    """
    n = -(-C // 512)
    while C % n or (C // n) % 2:
        n += 1
    w = C // n
    assert w >= 256, C
    return [(i * w, w) for i in range(n)]


def _build_nc(C):
    nc = bacc.Bacc("TRN2", target_bir_lowering=False, debug=False, num_devices=E)

    x_d = nc.dram_tensor("x_t", [P, KO1, C], f32, kind="ExternalInput").ap()
    w1_d = nc.dram_tensor("w1_t", [FC, P, KO1, P], f32, kind="ExternalInput").ap()
    b1_d = nc.dram_tensor("b1_t", [P, FC], f32, kind="ExternalInput").ap()
    w2_d = nc.dram_tensor("w2_t", [HO, P, KO2, P], f32, kind="ExternalInput").ap()
    b2_d = nc.dram_tensor("b2_t", [P, HO], f32, kind="ExternalInput").ap()
    g_d = nc.dram_tensor("g_t", [P, C], f32, kind="ExternalInput").ap()
    y_d = nc.dram_tensor("y", [H, C], f32, kind="ExternalOutput").ap()

    chunks = _c_chunks(C)

    with tile.TileContext(nc) as tc:
        with (
            tc.tile_pool(name="resident", bufs=1) as resident,
            tc.tile_pool(name="w1p", bufs=6) as w1p,
            tc.tile_pool(name="w2p", bufs=3) as w2p,
            tc.tile_pool(name="outp", bufs=3) as outp,
            tc.tile_pool(name="ps1", bufs=4, space="PSUM") as ps1,
            tc.tile_pool(name="ps2", bufs=3, space="PSUM") as ps2,
        ):
            # Warm the PE HAM clock-gate during the DMA fill window:
            # ~7us of matmuls on zeroed tiles unthrottles the clock
            # (1.2 -> 2.4 GHz needs ~3.4us of sustained PE activity), so
            # the real fc1 stream starts warm. Results are discarded.
            bf16 = mybir.dt.bfloat16
            wz_s = resident.tile([P, P], bf16, tag="wz_s")
            wz_m = resident.tile([P, 512], bf16, tag="wz_m")
            nc.vector.memset(wz_s[:], 0)
            nc.vector.memset(wz_m[:], 0)
            ps_w = ps1.tile([P, 512], f32, tag="ps", name="ps_warm")
            for i in range(14):
                nc.tensor.matmul(
                    ps_w[:], wz_s[:], wz_m[:], start=(i == 0), stop=(i == 13)
                )

            # The first fc1 matmuls are the critical path out of the
            # preamble: split w1[0] per-k and x[0] per-chunk so matmul (k=0,
            # chunk 0) waits on ~200KB instead of ~3MB.
            w1_first = [
                w1p.tile([P, P], f32r, tag=f"w1f{k}", name=f"w1f{k}")
                for k in range(KO1)
            ]
            x0_sb = []
            for c0, cw in _c_chunks(C):
                x0_sb.append(
                    resident.tile([P, cw], f32r, tag=f"x0_{c0}", name=f"x0_{c0}")
                )
            # issue order = time-to-first-matmul order: (k=0 weights, chunk-0
            # rhs) first, then the rest of iteration 0's operands
            nc.sync.dma_start(w1_first[0][:], w1_d[0, :, 0, :].bitcast(f32r))
            for ci, (c0, cw) in enumerate(_c_chunks(C)):
                nc.sync.dma_start(x0_sb[ci][:], x_d[:, 0, c0 : c0 + cw].bitcast(f32r))
            for k in range(1, KO1):
                nc.sync.dma_start(w1_first[k][:], w1_d[0, :, k, :].bitcast(f32r))
            # remaining x slices per-ko
            x_sb = [None]
            for k in range(1, KO1):
                xk = resident.tile([P, C], f32r, tag=f"x{k}")
                nc.sync.dma_start(xk[:], x_d[:, k, :].bitcast(f32r))
                x_sb.append(xk)
            b1_sb = resident.tile([P, FC], f32)
            nc.sync.dma_start(b1_sb[:], b1_d)
            b2_sb = resident.tile([P, HO], f32)
            nc.gpsimd.dma_start(b2_sb[:], b2_d)
            g_sb = resident.tile([P, C], f32)
            nc.gpsimd.dma_start(g_sb[:], g_d)
            h_sb = resident.tile([P, KO2, C], f32r)

            # fc2 weights: four 0.5MB DMAs per output h-tile. Each load is
            # anchored (real dependency edge) to a chosen fc1 gelu so the
            # scheduler cannot hoist the bulk w2 traffic into the early fc1
            # window where it would starve the w1 stream.
            w2_tiles: dict[int, object] = {}

            def load_w2(ho, anchor=None):
                if ho in w2_tiles or ho >= HO:
                    return
                t = w2p.tile([P, KO2, P], f32r, tag="w2")
                for g in range(4):
                    bi = nc.sync.dma_start(
                        t[:, g * 8 : (g + 1) * 8, :],
                        w2_d[ho][:, g * 8 : (g + 1) * 8, :].bitcast(f32r),
                    )
                    if anchor is not None:
                        add_dep_helper(
                            bi.ins, anchor.ins, reason="pace w2 prefetch vs fc1"
                        )
                w2_tiles[ho] = t

            # fc1: hT[f, c] = gelu(sum_k w1T[k, f] * xT[k, c] + b1[f])
            for fc in range(FC):
                if fc == 0:
                    w1_sb = None
                else:
                    w1_sb = w1p.tile([P, KO1, P], f32r)
                    nc.sync.dma_start(w1_sb[:], w1_d[fc].bitcast(f32r))
                for ci, (c0, cw) in enumerate(chunks):
                    ps = ps1.tile([P, 512], f32)
                    for k in range(KO1):
                        if fc == 0:
                            lhsT = w1_first[k][:]
                            rhs = (
                                x0_sb[ci][:, :]
                                if k == 0
                                else x_sb[k][:, c0 : c0 + cw]
                            )
                        else:
                            lhsT = w1_sb[:, k, :]
                            rhs = (
                                x0_sb[ci][:, :]
                                if k == 0
                                else x_sb[k][:, c0 : c0 + cw]
                            )
                        nc.tensor.matmul(
                            ps[:, :cw],
                            lhsT,
                            rhs,
                            start=(k == 0),
                            stop=(k == KO1 - 1),
                        )
                    act = nc.scalar.activation(
                        h_sb[:, fc, c0 : c0 + cw],
                        ps[:, :cw],
                        mybir.ActivationFunctionType.Gelu,
                        bias=b1_sb[:, fc : fc + 1],
                        scale=1.0,
                    )
                anchors = {10: 0, 20: 1, 27: 2, 31: 3}
                if fc in anchors:
                    load_w2(anchors[fc], anchor=act)

            # fc2: yT[h, c] = (sum_k w2T[k, h] * hT[k, c] + b2[h]) * g[c]
            for ho in range(HO):
                load_w2(ho)
                load_w2(ho + 4)  # slot-gated just-in-time prefetch
                w2_sb = w2_tiles[ho]
                for c0, cw in chunks:
                    ps = ps2.tile([P, 512], f32)
                    for k in range(KO2):
                        nc.tensor.matmul(
                            ps[:, :cw],
                            w2_sb[:, k, :],
                            h_sb[:, k, c0 : c0 + cw],
                            start=(k == 0),
                            stop=(k == KO2 - 1),
                        )
                    o1 = outp.tile([P, 512], f32, tag="o1")
                    nc.scalar.activation(
                        o1[:, :cw],
                        ps[:, :cw],
                        mybir.ActivationFunctionType.Identity,
                        bias=b2_sb[:, ho : ho + 1],
                        scale=1.0,
                    )
                    o2 = outp.tile([P, 512], f32, tag="o2")
                    nc.vector.tensor_mul(o2[:, :cw], o1[:, :cw], g_sb[:, c0 : c0 + cw])
                    nc.scalar.dma_start(
                        y_d[ho * P : (ho + 1) * P, c0 : c0 + cw], o2[:, :cw]
                    )

    nc.compile()
    return nc


def kernel(x, router_w, fc1_w, fc1_b, fc2_w, fc2_b):
    global LAST_RESULTS
    x = np.ascontiguousarray(np.asarray(x, dtype=np.float32))
    router_w = np.asarray(router_w, dtype=np.float32)
    B, S, _ = x.shape
    T = B * S
    xf = x.reshape(T, H)

    # --- host routing control plane (top-1 dispatch) ---
    logits = xf @ router_w.T  # [T, E] f32
    m = logits.max(-1, keepdims=True)
    ex = np.exp(logits - m, dtype=np.float32)
    denom = ex.sum(-1, dtype=np.float32)
    gates = np.float32(1.0) / denom  # max prob = exp(m - m) / denom
    idx = logits.argmax(-1)
    counts = np.bincount(idx, minlength=E)
    C = max(256, int(-(-counts.max() // 64) * 64))

    perm = [np.nonzero(idx == e)[0] for e in range(E)]

    if C not in _NC_CACHE:
        _NC_CACHE[C] = _build_nc(C)
    nc = _NC_CACHE[C]

    in_maps = []
    for e in range(E):
        pe = perm[e]
        ne = len(pe)
        xg = np.zeros((C, H), dtype=np.float32)
        xg[:ne] = xf[pe]
        x_t = np.ascontiguousarray(xg.T.reshape(KO1, P, C).transpose(1, 0, 2))
        w1 = np.asarray(fc1_w[e], dtype=np.float32)  # [F, H]
        w1_t = np.ascontiguousarray(
            w1.reshape(FC, P, KO1, P).transpose(0, 3, 2, 1)
        )  # [fc, kp, ko, fi]
        b1_t = np.ascontiguousarray(
            np.asarray(fc1_b[e], dtype=np.float32).reshape(FC, P).T
        )
        w2 = np.asarray(fc2_w[e], dtype=np.float32)  # [H, F]
        w2_t = np.ascontiguousarray(
            w2.reshape(HO, P, KO2, P).transpose(0, 3, 2, 1)
        )  # [ho, kp, ko, hi]
        b2_t = np.ascontiguousarray(
            np.asarray(fc2_b[e], dtype=np.float32).reshape(HO, P).T
        )
        ge = np.zeros(C, dtype=np.float32)
        ge[:ne] = gates[pe]
        g_t = np.ascontiguousarray(np.broadcast_to(ge, (P, C)))
        in_maps.append(
            {
                "x_t": x_t,
                "w1_t": w1_t,
                "b1_t": b1_t,
                "w2_t": w2_t,
                "b2_t": b2_t,
                "g_t": g_t,
            }
        )

    res = run_bass_kernel_spmd(nc, in_maps, core_ids=list(range(E)))
    LAST_RESULTS = res

    out_flat = np.empty((T, H), dtype=np.float32)
    for e in range(E):
        pe = perm[e]
        out_flat[pe] = res.results[e]["y"].T[: len(pe)]
    output = out_flat.reshape(B, S, H)

    # load-balance aux loss (scalar; part of the routing control plane)
    probs_sum = (ex / denom[:, None]).sum(axis=0, dtype=np.float64)
    mean_probs = probs_sum / T
    freq = counts.astype(np.float64) / T
    aux_loss = np.float32(LB_WEIGHT * E * np.sum(mean_probs * freq))
    return output, aux_loss
